# revision 45
# baseline (speedup 1.0000x reference)
"""Trainium2 Bass kernel for 3x3 VALID conv: x[32,128,64,64] * w[256,128,3,3] + bias.

Strategy:
  - Data-parallel over batch: 8 cores x 4 images each; weights/bias replicated.
  - Per core: implicit GEMM. Contraction dim = C_IN = 128 = partition dim.
    For each filter tap (u,v), accumulate
        psum[o, i, j] += W[c, o; u,v].T @ x[c, i+u, j+v]
    with the moving operand a strided [r, 62] view of a [C, rows, W] input
    piece, so only the 62 valid output columns are streamed.
  - bf16 x/w/y (accuracy ~4e-3 << 2e-2 tol): halves DMA bytes and SBUF
    bandwidth; matmul still 1 cycle/row. PSUM accumulates fp32.
  - Critical-path fusion: the host packs [x image0 rows 0-9 | half0 taps |
    half1 taps 6-8] into one DRAM tensor loaded by ONE DMA as the Sync
    ring's first batch (~10.7us); half1 taps 0-5 (needed ~2.5us later) are
    the Scalar ring's first batch. Tile dependencies are tile-granular and
    a ring's 2nd DMA batch lands ~3.5us late, so only first batches carry
    critical data.
  - Dummy matmuls on a zeroed scratch tile ramp the PE DVFS p-state until
    that DMA lands; a stream gap would reset the ramp to half clock.
  - Rings: fused xw + y stores on Sync (fast queue class); x piece prefetch
    and bias on Scalar (latency-tolerant).
  - The very last PSUM group is split in two so the first sub-group's
    evacuation + store hide under the second's matmuls (shorter tail).
"""

import numpy as np
import ml_dtypes

import concourse.bacc as bacc
import concourse.tile as tile
from concourse import mybir
from concourse.bass_utils import run_bass_kernel_spmd

N_CORES = 8
B_FULL, C_IN, H, W = 32, 128, 64, 64
C_OUT, KH, KW = 256, 3, 3
B_LOC = B_FULL // N_CORES          # images per core
H_OUT = W_OUT = H - KH + 1         # 62
N_HALF = C_OUT // 128              # 2 output-channel halves
RPC = 8                            # output rows per PSUM chunk (one bank)
N_CHUNKS = (H_OUT + RPC - 1) // RPC
P_ROWS = 2 * RPC + KH - 1          # input rows per 2-chunk x piece (18)
N_PIECES = 4                       # pieces [0:18],[16:34],[32:50],[48:64]
N_WARM = 8                         # DVFS warm-up matmuls
X0_ROWS = RPC + KH - 1             # fused tile: x image0 rows 0-9 ...
X0_LEN = X0_ROWS * W
W_LEN = N_HALF * KH * KW * 128
H1A = 6                            # half1 taps 0-5 ride the Scalar ring
WF_LEN = (KH * KW + KH * KW - H1A) * 128   # ... h0 taps + h1 taps 6-8
XW_LEN = X0_LEN + WF_LEN

_cached = {}


def _build_nc():
    f32 = mybir.dt.float32
    bf16 = mybir.dt.bfloat16
    nc = bacc.Bacc()

    xw_d = nc.declare_dram_parameter("xw0", [C_IN, XW_LEN], bf16, isOutput=False)
    w1a_d = nc.declare_dram_parameter("w1a", [C_IN, H1A, 128], bf16, isOutput=False)
    x_d = nc.declare_dram_parameter("x", [B_LOC, C_IN, H, W], bf16, isOutput=False)
    b_d = nc.declare_dram_parameter("bias_in", [128, N_HALF], f32, isOutput=False)
    y_d = nc.declare_dram_parameter(
        "y", [B_LOC, N_HALF, 128, H_OUT, W_OUT], bf16, isOutput=True
    )

    with tile.TileContext(nc) as tc:
        with (
            tc.tile_pool(name="const", bufs=1) as cpool,
            tc.tile_pool(name="xin", bufs=5) as xpool,
            tc.tile_pool(name="out", bufs=4) as opool,
            tc.tile_pool(name="psum", bufs=4, space="PSUM") as ppool,
            tc.tile_pool(name="warm", bufs=1, space="PSUM") as wpool,
        ):
            ct = cpool.tile([C_IN, XW_LEN], bf16)
            w1a_t = cpool.tile([C_IN, H1A, 128], bf16)
            b_t = cpool.tile([128, N_HALF], f32)
            scr = cpool.tile([128, 512], bf16)

            nc.vector.memset(scr[:], 0.0)

            # Critical-path DMAs: the slim fused tile (x piece0 + h0 taps +
            # h1 taps 6-8) is the Sync ring's first batch; h1 taps 0-5 are
            # the Scalar ring's first batch (needed ~2.5us after stream
            # start, and a ring's FIRST batch lands early).
            nc.sync.dma_start(ct[:], xw_d[:])
            nc.scalar.dma_start(w1a_t[:], w1a_d[:])
            nc.scalar.dma_start(b_t[:], b_d[:])

            x0v = ct[:, 0:X0_LEN].rearrange("c (h w) -> c h w", w=W)
            wv0 = ct[:, X0_LEN : X0_LEN + KH * KW * 128].rearrange(
                "c (t o) -> c t o", t=KH * KW
            )
            wv1b = ct[:, X0_LEN + KH * KW * 128 : XW_LEN].rearrange(
                "c (t o) -> c t o", t=KH * KW - H1A
            )

            def lhsT(half, uv):
                if half == 0:
                    return wv0[:, uv, :]
                return w1a_t[:, uv, :] if uv < H1A else wv1b[:, uv - H1A, :]

            # Ramp the PE p-state while the fused DMA lands (distinct sizes
            # so no two warm-up matmuls are identical instructions).
            pwarm = wpool.tile([128, 512], f32)
            for i in range(N_WARM):
                nc.tensor.matmul(
                    pwarm[:, 0 : 512 - i],
                    scr[:, 0:128],
                    scr[:, 0 : 512 - i],
                    start=True,
                    stop=True,
                )

            def load_piece(b, r0, r1, eng):
                px = xpool.tile([C_IN, P_ROWS, W], bf16, tag="x")
                eng.dma_start(px[:, 0 : r1 - r0, :], x_d[b, :, r0:r1, :])
                return px

            for b in range(B_LOC):
                if b == 0:
                    # chunk0 reads the fused tile; chunk1's piece must land
                    # ~4us after stream start, so it rides Sync right behind
                    # the fused DMA. The rest prefetch on Scalar.
                    pa1 = load_piece(0, 8, 18, nc.sync)   # chunk1 only: slim,
                    pa2 = load_piece(0, 16, 34, nc.sync)  # lands ~0.3us earlier
                    pb = load_piece(0, 24, 42, nc.scalar)
                    pc = load_piece(0, 40, 58, nc.scalar)
                    pd = load_piece(0, 56, 64, nc.scalar)
                    chunk_map = [
                        (x0v, 0), (pa1, 0), (pa2, 0), (pb, 0),
                        (pb, 8), (pc, 0), (pc, 8), (pd, 0),
                    ]
                else:
                    piece_tiles = [
                        load_piece(b, 2 * RPC * k, min(2 * RPC * k + P_ROWS, H),
                                   nc.scalar)
                        for k in range(N_PIECES)
                    ]
                    chunk_map = [
                        (piece_tiles[c // 2], (c % 2) * RPC)
                        for c in range(N_CHUNKS)
                    ]
                def do_group(px, li, b, half, i0, r, store_eng=nc.sync):
                    ps = ppool.tile([128, RPC, W_OUT], f32, tag="ps")
                    for uv in range(KH * KW):
                        u, v = divmod(uv, KW)
                        nc.tensor.matmul(
                            ps[:, 0:r, :],
                            lhsT(half, uv),
                            px[:, li + u : li + u + r, v : v + W_OUT],
                            start=(uv == 0),
                            stop=(uv == KH * KW - 1),
                        )
                    o_t = opool.tile([128, RPC, W_OUT], bf16, tag="o")
                    nc.vector.tensor_scalar_add(
                        o_t[:, 0:r, :], ps[:, 0:r, :], b_t[:, half : half + 1]
                    )
                    store_eng.dma_start(
                        y_d[b, half, :, i0 : i0 + r, :], o_t[:, 0:r, :]
                    )

                for chunk in range(N_CHUNKS):
                    i0 = chunk * RPC
                    r = min(RPC, H_OUT - i0)
                    px, li = chunk_map[chunk]
                    for half in range(N_HALF):
                        last = (b == B_LOC - 1 and chunk == N_CHUNKS - 1
                                and half == N_HALF - 1)
                        if last:
                            # Split the very last PSUM group so the first
                            # sub-group's evac + store hide under the second
                            # sub-group's matmuls, shrinking the kernel tail.
                            h1 = r - 2
                            # Sub-A's store rides Scalar so sub-B's final
                            # store isn't queued behind it on the Sync seq.
                            do_group(px, li, b, half, i0, h1,
                                     store_eng=nc.scalar)
                            do_group(px, li + h1, b, half, i0 + h1, r - h1)
                        else:
                            do_group(px, li, b, half, i0, r)

    nc.compile()
    if not nc.is_finalized():
        nc.finalize()
    return nc


def kernel(inputs, weights, bias, profile=False, trace_kwargs=None):
    x_b = np.ascontiguousarray(
        np.asarray(inputs, dtype=np.float32).astype(ml_dtypes.bfloat16)
    )
    # [O, C, KH, KW] -> [C, half, KH*KW, o_local]  (lhsT layout: contraction dim
    # on partitions; each half contiguous per partition for fast DMA)
    w_t = np.ascontiguousarray(
        np.asarray(weights, dtype=np.float32)
        .reshape(N_HALF, 128, C_IN, KH * KW)
        .transpose(2, 0, 3, 1)
        .astype(ml_dtypes.bfloat16)
    )
    w_flat = w_t.reshape(C_IN, W_LEN)
    wf = np.ascontiguousarray(
        np.concatenate(
            [w_t[:, 0].reshape(C_IN, -1), w_t[:, 1, H1A:].reshape(C_IN, -1)],
            axis=1,
        )
    )
    w1a_arr = np.ascontiguousarray(w_t[:, 1, 0:H1A])
    # [C_OUT, 1] -> [128, N_HALF] with bias_sb[p, h] = bias[h*128 + p]
    b_t = np.ascontiguousarray(
        np.asarray(bias, dtype=np.float32).reshape(N_HALF, 128).T
    )

    if "nc" not in _cached:
        _cached["nc"] = _build_nc()
    nc = _cached["nc"]

    in_maps = []
    for i in range(N_CORES):
        shard = x_b[i * B_LOC : (i + 1) * B_LOC]
        xw0 = np.ascontiguousarray(
            np.concatenate(
                [shard[0, :, 0:X0_ROWS, :].reshape(C_IN, X0_LEN), wf], axis=1
            )
        )
        in_maps.append({"xw0": xw0, "w1a": w1a_arr, "x": shard, "bias_in": b_t})
    res = run_bass_kernel_spmd(
        nc,
        in_maps,
        list(range(N_CORES)),
        trace=profile,
        **(trace_kwargs or {}),
    )
    _cached["last_result"] = res

    shards = []
    for i in range(N_CORES):
        y = res.results[i]["y"]  # [B_LOC, 2, 128, 62, 62] bf16
        shards.append(
            np.asarray(y).astype(np.float32).reshape(B_LOC, C_OUT, H_OUT, W_OUT)
        )
    return np.ascontiguousarray(np.concatenate(shards, axis=0), dtype=np.float32)
